# revision 3
# baseline (speedup 1.0000x reference)
"""Distributed Trainium2 kernel for ANEMultiHeadAttention.

Problem: B=2, C=1024, S=2048, H=16, D=64.
  x: (B, C, 1, S);  q = Wq x + bq; k = Wk x; v = Wv x + bv
  per-head attention (softmax over keys), out = Wo o + bo.

Sharding (8 cores): core i handles batch b = i // 4 and head-group
hg = i % 4 (4 heads = 256 channels). Q/K/V column-parallel, Wo
row-parallel; host sums the 4 partial outputs per batch.
The v-bias contributes Wo @ bv (softmax rows sum to 1) and is folded
into a host-side constant along with bo.

Per-core device algorithm (all matmuls bf16, f32 PSUM):
  - q = WqT_s^T @ x   (256, S)  [+bq, via DVE per-partition scalar add]
  - k = WkT_s^T @ x   (256, S)
  - vT = x^T @ WvT_s  (S, 256)  stored (128, st, head, 65) with a ones col
  - per head pair (A, B), per q-window of 1024:
      for kt in 16:  scoresT = k_kt^T q_win (row-packed A/B), exp (ACT,
      scale=1/8) -> bf16, PV: o_aug += vT_aug^T @ expT  (65 rows: 64 of
      o plus the denominator row)
      normalize: recip(denom) -> partition_broadcast -> multiply -> o bf16
  - out_partial = WoT_s^T @ o  (1024, S) f32 -> DMA out.
"""

import sys

for p in ("/opt/trn_rl_repo",):
    if p not in sys.path:
        sys.path.insert(0, p)

from contextlib import ExitStack

import ml_dtypes
import numpy as np

import concourse.bass as bass
import concourse.mybir as mybir
import concourse.tile as tile
from concourse import bacc
from concourse.bass_utils import run_bass_kernel_spmd

# Problem shape (hardcoded per contest rules)
B, C, S, H = 2, 1024, 2048, 16
D = C // H  # 64
N_CORES = 8
HG = 4  # head groups
HPG = H // HG  # heads per group = 4
CPG = HPG * D  # channels per group = 256
P = 128
NK = C // P  # 8 contraction tiles over C
NST = S // P  # 16 key tiles
WIN = 1024  # q window
NWIN = S // WIN  # 2
NCH = WIN // 512  # 512-chunks per window = 2

F32 = mybir.dt.float32
BF16 = mybir.dt.bfloat16
EXP = mybir.ActivationFunctionType.Exp

_CACHED_NC = None


def build_nc():
    nc = bacc.Bacc("TRN2", target_bir_lowering=False, debug=False)

    x_d = nc.dram_tensor("x", (P, NK, S), BF16, kind="ExternalInput")
    wq_d = nc.dram_tensor("wqT", (P, NK, CPG), BF16, kind="ExternalInput")
    wk_d = nc.dram_tensor("wkT", (P, NK, CPG), BF16, kind="ExternalInput")
    wv_d = nc.dram_tensor("wvT", (P, NK, CPG), BF16, kind="ExternalInput")
    wo_d = nc.dram_tensor("woT", (P, 2, C), BF16, kind="ExternalInput")
    bq_d = nc.dram_tensor("bq", (P, 2), F32, kind="ExternalInput")
    out_d = nc.dram_tensor("out", (P, NK, S), F32, kind="ExternalOutput")

    with tile.TileContext(nc) as tc, ExitStack() as ctx:
        const = ctx.enter_context(tc.tile_pool(name="const", bufs=1))
        work = ctx.enter_context(tc.tile_pool(name="work", bufs=1))
        expp = ctx.enter_context(tc.tile_pool(name="expp", bufs=3))
        outp = ctx.enter_context(tc.tile_pool(name="outp", bufs=3))
        smal = ctx.enter_context(tc.tile_pool(name="smal", bufs=4))
        # PSUM: pool_sc 2 tiles of 2 banks (QKV + scores), pool_pv same
        # (PV accumulators + out-proj) -> 8 banks total.
        psc = ctx.enter_context(tc.tile_pool(name="psc", bufs=2, space="PSUM"))
        ppv = ctx.enter_context(tc.tile_pool(name="ppv", bufs=2, space="PSUM"))

        # ---- constants / inputs in SBUF ----
        x_sb = const.tile([P, NK, S], BF16, tag="x")
        nc.sync.dma_start(x_sb[:], x_d[:])
        wq_sb = const.tile([P, NK, CPG], BF16, tag="wq")
        nc.sync.dma_start(wq_sb[:], wq_d[:])
        wk_sb = const.tile([P, NK, CPG], BF16, tag="wk")
        nc.sync.dma_start(wk_sb[:], wk_d[:])
        wv_sb = const.tile([P, NK, CPG], BF16, tag="wv")
        nc.sync.dma_start(wv_sb[:], wv_d[:])
        wo_sb = const.tile([P, 2, C], BF16, tag="wo")
        nc.sync.dma_start(wo_sb[:], wo_d[:])
        bq_sb = const.tile([P, 2], F32, tag="bq")
        nc.sync.dma_start(bq_sb[:], bq_d[:])

        # persistent activations
        q_sb = work.tile([P, 2, S], BF16, tag="q")
        k_sb = work.tile([P, 2, S], BF16, tag="k")
        vt_sb = work.tile([P, NST, HPG, D + 1], BF16, tag="vt")
        o_sb = work.tile([P, 2, S], BF16, tag="o")

        # ones column for the denominator rows (memset whole thing first;
        # the v-copies below overwrite cols 0:64 of each head slot)
        nc.vector.memset(vt_sb[:], 1.0)

        # ---- Q / K projections: out (256, S) as 2 M-tiles ----
        for name, w_sb, dst, bias in (
            ("q", wq_sb, q_sb, True),
            ("k", wk_sb, k_sb, False),
        ):
            for m in range(2):  # M-tile = head pair
                for w in range(NWIN):
                    ps = psc.tile([P, WIN], F32, tag="psc")
                    for kt in range(NK):
                        for ch in range(NCH):
                            nc.tensor.matmul(
                                ps[:, ch * 512 : (ch + 1) * 512],
                                wq_sb if False else w_sb[:, kt, m * P : (m + 1) * P],
                                x_sb[:, kt, w * WIN + ch * 512 : w * WIN + (ch + 1) * 512],
                                start=(kt == 0),
                                stop=(kt == NK - 1),
                            )
                    if bias:
                        nc.vector.tensor_scalar_add(
                            dst[:, m, w * WIN : (w + 1) * WIN],
                            ps[:],
                            bq_sb[:, m : m + 1],
                        )
                    else:
                        nc.vector.tensor_copy(
                            dst[:, m, w * WIN : (w + 1) * WIN], ps[:]
                        )

        # ---- V projection, transposed: vT (S, 256) ----
        for st in range(NST):
            ps = psc.tile([P, WIN], F32, tag="psc")
            for kt in range(NK):
                nc.tensor.matmul(
                    ps[:, :CPG],
                    x_sb[:, kt, st * P : (st + 1) * P],
                    wv_sb[:, kt, :],
                    start=(kt == 0),
                    stop=(kt == NK - 1),
                )
            # scatter heads: psum (128, 4, 64) -> vt_sb[:, st, h, 0:64]
            nc.vector.tensor_copy(
                vt_sb[:, st, :, 0:D],
                ps[:, :CPG].rearrange("p (h d) -> p h d", h=HPG),
            )

        # ---- attention ----
        for pair in range(2):
            for w in range(NWIN):
                oa = ppv.tile([P, WIN], F32, tag="ppv")  # head A accum (use 0:65)
                ob = ppv.tile([P, WIN], F32, tag="ppv")  # head B accum
                for kt in range(NST):
                    sa = psc.tile([P, WIN], F32, tag="psc")
                    sb = psc.tile([P, WIN], F32, tag="psc")
                    for ch in range(NCH):
                        cs = slice(ch * 512, (ch + 1) * 512)
                        qs = slice(w * WIN + ch * 512, w * WIN + (ch + 1) * 512)
                        nc.tensor.matmul(
                            sa[:, cs],
                            k_sb[0:D, pair, kt * P : (kt + 1) * P],
                            q_sb[0:D, pair, qs],
                        )
                        nc.tensor.matmul(
                            sb[:, cs],
                            k_sb[D:P, pair, kt * P : (kt + 1) * P],
                            q_sb[D:P, pair, qs],
                        )
                    ea = expp.tile([P, WIN], BF16, tag="exp")
                    eb = expp.tile([P, WIN], BF16, tag="exp")
                    nc.scalar.activation(ea[:], sa[:], EXP, scale=float(D) ** -0.5)
                    nc.scalar.activation(eb[:], sb[:], EXP, scale=float(D) ** -0.5)
                    for ch in range(NCH):
                        cs = slice(ch * 512, (ch + 1) * 512)
                        nc.tensor.matmul(
                            oa[0 : D + 1, cs],
                            vt_sb[:, kt, 2 * pair, :],
                            ea[:, cs],
                            start=(kt == 0),
                            stop=(kt == NST - 1),
                        )
                        nc.tensor.matmul(
                            ob[0 : D + 1, cs],
                            vt_sb[:, kt, 2 * pair + 1, :],
                            eb[:, cs],
                            start=(kt == 0),
                            stop=(kt == NST - 1),
                        )
                # normalize: o = o / denom  (denom = row 64)
                for head, acc in ((0, oa), (1, ob)):
                    rc = smal.tile([1, WIN], F32, tag="rc")
                    nc.vector.reciprocal(rc[:], acc[D : D + 1, :])
                    rcb = smal.tile([D, WIN], F32, tag="rcb")
                    nc.gpsimd.partition_broadcast(rcb[:], rc[:])
                    nc.vector.tensor_mul(
                        o_sb[head * D : (head + 1) * D, pair, w * WIN : (w + 1) * WIN],
                        acc[0:D, :],
                        rcb[:],
                    )

        # ---- output projection: out_partial (1024, S) f32 ----
        for m in range(NK):
            ot = outp.tile([P, S], F32, tag="ot")
            for w in range(NWIN):
                ps = ppv.tile([P, WIN], F32, tag="ppv")
                for kt in range(2):
                    for ch in range(NCH):
                        cs = slice(ch * 512, (ch + 1) * 512)
                        nc.tensor.matmul(
                            ps[:, cs],
                            wo_sb[:, kt, m * P : (m + 1) * P],
                            o_sb[:, kt, w * WIN + ch * 512 : w * WIN + (ch + 1) * 512],
                            start=(kt == 0),
                            stop=(kt == 1),
                        )
                nc.vector.tensor_copy(ot[:, w * WIN : (w + 1) * WIN], ps[:])
            nc.sync.dma_start(out_d[:, m, :], ot[:])

    nc.compile()
    return nc


def _shard_inputs(hidden_states, Wq, bq, Wk, Wv, bv, Wo, bo):
    bf = ml_dtypes.bfloat16
    in_maps = []
    for core in range(N_CORES):
        b, hg = divmod(core, HG)
        x = hidden_states[b, :, 0, :]  # (C, S) f32
        cs = slice(hg * CPG, (hg + 1) * CPG)
        wqT = Wq[cs, :].T.reshape(NK, P, CPG).transpose(1, 0, 2)
        wkT = Wk[cs, :].T.reshape(NK, P, CPG).transpose(1, 0, 2)
        wvT = Wv[cs, :].T.reshape(NK, P, CPG).transpose(1, 0, 2)
        woT = Wo[:, cs].T.reshape(2, P, C).transpose(1, 0, 2)
        in_maps.append(
            {
                "x": np.ascontiguousarray(
                    x.reshape(NK, P, S).transpose(1, 0, 2)
                ).astype(bf),
                "wqT": np.ascontiguousarray(wqT).astype(bf),
                "wkT": np.ascontiguousarray(wkT).astype(bf),
                "wvT": np.ascontiguousarray(wvT).astype(bf),
                "woT": np.ascontiguousarray(woT).astype(bf),
                "bq": np.ascontiguousarray(
                    bq[cs].reshape(2, P).T
                ).astype(np.float32),
            }
        )
    return in_maps


def get_nc():
    global _CACHED_NC
    if _CACHED_NC is None:
        _CACHED_NC = build_nc()
    return _CACHED_NC


def run(hidden_states, Wq, bq, Wk, Wv, bv, Wo, bo, trace=False, **kw):
    nc = get_nc()
    in_maps = _shard_inputs(hidden_states, Wq, bq, Wk, Wv, bv, Wo, bo)
    res = run_bass_kernel_spmd(
        nc, in_maps, core_ids=list(range(N_CORES)), trace=trace, **kw
    )
    # unshard: sum partials per batch, add host-side constant bias
    bias_vec = (Wo.astype(np.float64) @ bv.astype(np.float64)).astype(
        np.float32
    ) + bo
    out = np.zeros((B, C, 1, S), dtype=np.float32)
    for core in range(N_CORES):
        b = core // HG
        part = np.asarray(res.results[core]["out"], dtype=np.float32)
        out[b, :, 0, :] += part.transpose(1, 0, 2).reshape(C, S)
    out[:, :, 0, :] += bias_vec[None, :, None]
    return out, res


def kernel(**inputs):
    out, _ = run(**inputs)
    return out


# revision 6
# speedup vs baseline: 1.0635x; 1.0635x over previous
"""Distributed Trainium2 kernel for ANEMultiHeadAttention.

Problem: B=2, C=1024, S=2048, H=16, D=64.
  x: (B, C, 1, S);  q = Wq x + bq; k = Wk x; v = Wv x + bv
  per-head attention (softmax over keys), out = Wo o + bo.

Sharding (8 cores): core i handles batch b = i // 4 and head-group
hg = i % 4 (4 heads = 256 channels). Q/K/V column-parallel, Wo
row-parallel; host sums the 4 partial outputs per batch.
The v-bias contributes Wo @ bv (softmax rows sum to 1) and is folded
into a host-side constant along with bo.

Per-core device algorithm (all matmuls bf16, f32 PSUM):
  - q = WqT_s^T @ x   (256, S)  [+bq, via DVE per-partition scalar add]
  - k = WkT_s^T @ x   (256, S)
  - vT = x^T @ WvT_s  (S, 256)  stored (128, head, 65) per key-tile,
    with a ones column (PV then also accumulates softmax denominators)
  - per q-window of 1024, per head pair (A, B):
      for kt in 16: scoresT = k_kt^T q_win (row-packed A/B via
      tile_position), exp (ACT, scale=1/8) -> bf16,
      PV: o_aug += vT_aug^T @ expT  (65 rows: 64 of o + denominator)
      evac o_aug -> SBUF fast (frees PSUM), then normalize off the
      critical path: recip_approx -> partition_broadcast -> multiply
  - out_partial = WoT_s^T @ o per window (overlaps next window) -> DMA.
"""

import sys

for p in ("/opt/trn_rl_repo",):
    if p not in sys.path:
        sys.path.insert(0, p)

from contextlib import ExitStack

import ml_dtypes
import numpy as np

import concourse.bass as bass
import concourse.mybir as mybir
import concourse.tile as tile
from concourse import bacc
from concourse.bass_utils import run_bass_kernel_spmd

# Problem shape (hardcoded per contest rules)
B, C, S, H = 2, 1024, 2048, 16
D = C // H  # 64
N_CORES = 8
HG = 4  # head groups
HPG = H // HG  # heads per group = 4
CPG = HPG * D  # channels per group = 256
P = 128
NK = C // P  # 8 contraction tiles over C
NST = S // P  # 16 key tiles
WIN = 1024  # q window
NWIN = S // WIN  # 2
NCH = WIN // 512  # 512-chunks per window = 2

F32 = mybir.dt.float32
BF16 = mybir.dt.bfloat16
EXP = mybir.ActivationFunctionType.Exp

_CACHED_NC = None


def build_nc():
    nc = bacc.Bacc("TRN2", target_bir_lowering=False, debug=False)

    x_d = nc.dram_tensor("x", (P, NK, S), BF16, kind="ExternalInput")
    wq_d = nc.dram_tensor("wqT", (P, NK, CPG), BF16, kind="ExternalInput")
    wk_d = nc.dram_tensor("wkT", (P, NK, CPG), BF16, kind="ExternalInput")
    wv_d = nc.dram_tensor("wvT", (P, NK, CPG), BF16, kind="ExternalInput")
    wo_d = nc.dram_tensor("woT", (P, 2, C), BF16, kind="ExternalInput")
    bq_d = nc.dram_tensor("bq", (P, 2), F32, kind="ExternalInput")
    out_d = nc.dram_tensor("out", (P, NK, S), F32, kind="ExternalOutput")

    with tile.TileContext(nc) as tc, ExitStack() as ctx:
        const = ctx.enter_context(tc.tile_pool(name="const", bufs=1))
        work = ctx.enter_context(tc.tile_pool(name="work", bufs=1))
        expp = ctx.enter_context(tc.tile_pool(name="expp", bufs=8))
        onp = ctx.enter_context(tc.tile_pool(name="onp", bufs=4))
        outp = ctx.enter_context(tc.tile_pool(name="outp", bufs=3))
        smal = ctx.enter_context(tc.tile_pool(name="smal", bufs=4))
        psc = ctx.enter_context(tc.tile_pool(name="psc", bufs=2, space="PSUM"))
        ppv = ctx.enter_context(tc.tile_pool(name="ppv", bufs=2, space="PSUM"))

        # ---- inputs in SBUF; x split per k-tile so DMA pipelines ----
        xt = []
        for kt in range(NK):
            t = const.tile([P, S], BF16, tag=f"x{kt}")
            nc.sync.dma_start(t[:], x_d[:, kt, :])
            xt.append(t)
        wq_sb = const.tile([P, NK, CPG], BF16, tag="wq")
        nc.sync.dma_start(wq_sb[:], wq_d[:])
        wk_sb = const.tile([P, NK, CPG], BF16, tag="wk")
        nc.sync.dma_start(wk_sb[:], wk_d[:])
        wv_sb = const.tile([P, NK, CPG], BF16, tag="wv")
        nc.sync.dma_start(wv_sb[:], wv_d[:])
        wo_sb = const.tile([P, 2, C], BF16, tag="wo")
        nc.sync.dma_start(wo_sb[:], wo_d[:])
        bq_sb = const.tile([P, 2], F32, tag="bq")
        nc.sync.dma_start(bq_sb[:], bq_d[:])

        # persistent activations (separate tiles per pair for fine deps)
        q_sb = [
            work.tile([P, S], BF16, tag=f"q{p_}", name=f"q{p_}") for p_ in range(2)
        ]
        k_sb = [
            work.tile([P, S], BF16, tag=f"k{p_}", name=f"k{p_}") for p_ in range(2)
        ]
        vt = [
            work.tile([P, HPG, D + 1], BF16, tag=f"vt{st}", name=f"vt{st}")
            for st in range(NST)
        ]
        o_sb = [
            work.tile([P, S], BF16, tag=f"o{p_}", name=f"o{p_}") for p_ in range(2)
        ]

        def qk_proj(w_sb, dst, pair, w, bias):
            ps = psc.tile([P, WIN], F32, tag="psc")
            for kt in range(NK):
                for ch in range(NCH):
                    nc.tensor.matmul(
                        ps[:, ch * 512 : (ch + 1) * 512],
                        w_sb[:, kt, pair * P : (pair + 1) * P],
                        xt[kt][:, w * WIN + ch * 512 : w * WIN + (ch + 1) * 512],
                        start=(kt == 0),
                        stop=(kt == NK - 1),
                    )
            if bias:
                nc.vector.tensor_scalar_add(
                    dst[:, w * WIN : (w + 1) * WIN], ps[:], bq_sb[:, pair : pair + 1]
                )
            else:
                nc.vector.tensor_copy(dst[:, w * WIN : (w + 1) * WIN], ps[:])

        # QKV: pair-0 window-0 first (unblocks attention soonest)
        qk_proj(wk_sb, k_sb[0], 0, 0, False)
        qk_proj(wq_sb, q_sb[0], 0, 0, True)

        # V projection, transposed: vT (S, 256) per key-tile
        for st in range(NST):
            nc.vector.memset(vt[st][:], 1.0)
            ps = psc.tile([P, WIN], F32, tag="psc")
            for kt in range(NK):
                nc.tensor.matmul(
                    ps[:, :CPG],
                    xt[kt][:, st * P : (st + 1) * P],
                    wv_sb[:, kt, :],
                    start=(kt == 0),
                    stop=(kt == NK - 1),
                )
            nc.vector.tensor_copy(
                vt[st][:, :, 0:D],
                ps[:, :CPG].rearrange("p (h d) -> p h d", h=HPG),
            )

        qk_proj(wk_sb, k_sb[0], 0, 1, False)
        qk_proj(wq_sb, q_sb[0], 0, 1, True)
        for w in range(NWIN):
            qk_proj(wk_sb, k_sb[1], 1, w, False)
            qk_proj(wq_sb, q_sb[1], 1, w, True)

        # ---- attention + per-window output projection ----
        def attention(w, pair):
            oa = ppv.tile([P, WIN], F32, tag="ppv")
            ob = ppv.tile([P, WIN], F32, tag="ppv")
            for kt in range(NST):
                sa = psc.tile([P, WIN], F32, tag="psc")
                sb = psc.tile([P, WIN], F32, tag="psc")
                for ch in range(NCH):
                    cs = slice(ch * 512, (ch + 1) * 512)
                    qs = slice(w * WIN + ch * 512, w * WIN + (ch + 1) * 512)
                    nc.tensor.matmul(
                        sa[:, cs],
                        k_sb[pair][0:D, kt * P : (kt + 1) * P],
                        q_sb[pair][0:D, qs],
                        tile_position=(0, 0),
                    )
                    nc.tensor.matmul(
                        sb[:, cs],
                        k_sb[pair][D:P, kt * P : (kt + 1) * P],
                        q_sb[pair][D:P, qs],
                        tile_position=(64, 0),
                    )
                ea = expp.tile([P, WIN], BF16, tag="exp")
                eb = expp.tile([P, WIN], BF16, tag="exp")
                nc.scalar.activation(ea[:], sa[:], EXP, scale=float(D) ** -0.5)
                nc.scalar.activation(eb[:], sb[:], EXP, scale=float(D) ** -0.5)
                for ch in range(NCH):
                    cs = slice(ch * 512, (ch + 1) * 512)
                    nc.tensor.matmul(
                        oa[0 : D + 1, cs],
                        vt[kt][:, 2 * pair, :],
                        ea[:, cs],
                        start=(kt == 0),
                        stop=(kt == NST - 1),
                    )
                    nc.tensor.matmul(
                        ob[0 : D + 1, cs],
                        vt[kt][:, 2 * pair + 1, :],
                        eb[:, cs],
                        start=(kt == 0),
                        stop=(kt == NST - 1),
                    )
            # fast evac to SBUF (frees PSUM slots), normalize off-path
            for head, acc in ((0, oa), (1, ob)):
                ou = onp.tile([D + 1, WIN], F32, tag="ou")
                nc.vector.tensor_copy(ou[:], acc[0 : D + 1, :])
                rc = smal.tile([1, WIN], F32, tag="rc")
                nc.vector.reciprocal(rc[:], ou[D : D + 1, :])
                rcb = smal.tile([D, WIN], F32, tag="rcb")
                nc.gpsimd.partition_broadcast(rcb[:], rc[:])
                nc.vector.tensor_mul(
                    o_sb[pair][head * D : (head + 1) * D, w * WIN : (w + 1) * WIN],
                    ou[0:D, :],
                    rcb[:],
                )

        def outproj(w):
            for m in range(NK):
                ps = ppv.tile([P, WIN], F32, tag="ppv")
                for kt in range(2):
                    for ch in range(NCH):
                        cs = slice(ch * 512, (ch + 1) * 512)
                        nc.tensor.matmul(
                            ps[:, cs],
                            wo_sb[:, kt, m * P : (m + 1) * P],
                            o_sb[kt][:, w * WIN + ch * 512 : w * WIN + (ch + 1) * 512],
                            start=(kt == 0),
                            stop=(kt == 1),
                        )
                ot = outp.tile([P, WIN], F32, tag="ot")
                nc.vector.tensor_copy(ot[:], ps[:])
                nc.sync.dma_start(out_d[:, m, w * WIN : (w + 1) * WIN], ot[:])

        for w in range(NWIN):
            attention(w, 0)
            attention(w, 1)
            outproj(w)

    nc.compile()
    return nc


def _shard_inputs(hidden_states, Wq, bq, Wk, Wv, bv, Wo, bo):
    bf = ml_dtypes.bfloat16
    in_maps = []
    for core in range(N_CORES):
        b, hg = divmod(core, HG)
        x = hidden_states[b, :, 0, :]  # (C, S) f32
        cs = slice(hg * CPG, (hg + 1) * CPG)
        wqT = Wq[cs, :].T.reshape(NK, P, CPG).transpose(1, 0, 2)
        wkT = Wk[cs, :].T.reshape(NK, P, CPG).transpose(1, 0, 2)
        wvT = Wv[cs, :].T.reshape(NK, P, CPG).transpose(1, 0, 2)
        woT = Wo[:, cs].T.reshape(2, P, C).transpose(1, 0, 2)
        in_maps.append(
            {
                "x": np.ascontiguousarray(
                    x.reshape(NK, P, S).transpose(1, 0, 2)
                ).astype(bf),
                "wqT": np.ascontiguousarray(wqT).astype(bf),
                "wkT": np.ascontiguousarray(wkT).astype(bf),
                "wvT": np.ascontiguousarray(wvT).astype(bf),
                "woT": np.ascontiguousarray(woT).astype(bf),
                "bq": np.ascontiguousarray(
                    bq[cs].reshape(2, P).T
                ).astype(np.float32),
            }
        )
    return in_maps


def get_nc():
    global _CACHED_NC
    if _CACHED_NC is None:
        _CACHED_NC = build_nc()
    return _CACHED_NC


def run(hidden_states, Wq, bq, Wk, Wv, bv, Wo, bo, trace=False, **kw):
    nc = get_nc()
    in_maps = _shard_inputs(hidden_states, Wq, bq, Wk, Wv, bv, Wo, bo)
    res = run_bass_kernel_spmd(
        nc, in_maps, core_ids=list(range(N_CORES)), trace=trace, **kw
    )
    # unshard: sum partials per batch, add host-side constant bias
    bias_vec = (Wo.astype(np.float64) @ bv.astype(np.float64)).astype(
        np.float32
    ) + bo
    out = np.zeros((B, C, 1, S), dtype=np.float32)
    for core in range(N_CORES):
        b = core // HG
        part = np.asarray(res.results[core]["out"], dtype=np.float32)
        out[b, :, 0, :] += part.transpose(1, 0, 2).reshape(C, S)
    out[:, :, 0, :] += bias_vec[None, :, None]
    return out, res


def kernel(**inputs):
    out, _ = run(**inputs)
    return out


# revision 7
# speedup vs baseline: 1.0838x; 1.0191x over previous
"""Distributed Trainium2 kernel for ANEMultiHeadAttention.

Problem: B=2, C=1024, S=2048, H=16, D=64.
  x: (B, C, 1, S);  q = Wq x + bq; k = Wk x; v = Wv x + bv
  per-head attention (softmax over keys), out = Wo o + bo.

Sharding (8 cores): core i handles batch b = i // 4 and head-group
hg = i % 4 (4 heads = 256 channels). Q/K/V column-parallel, Wo
row-parallel; host sums the 4 partial outputs per batch.
The v-bias contributes Wo @ bv (softmax rows sum to 1) and is folded
into a host-side constant along with bo.

Per-core device algorithm (all matmuls bf16, f32 PSUM):
  - q = WqT_s^T @ x (+bq), k = WkT_s^T @ x, computed in (pair, 1024)
    units; vT = x^T @ WvT_s stored (128, head, 65) per key-tile with a
    ones column (PV then also accumulates softmax denominators).
  - attention per q-window of 1024, per head pair: scoresT = k^T q
    (row-packed pair via tile_position), exp on ACT (scale 1/8),
    PV: o_aug += vT_aug^T @ expT.  The exp stream on the Scalar engine
    is the critical path; QKV units are emission-interleaved into the
    first window so the PE stays dense (and HAM-warm) while ACT works.
  - normalize (recip + partition_broadcast + mul) off the critical
    path after a fast PSUM->SBUF evac; out-projection per window is
    deferred one attention block so it never stalls the exp stream.
"""

import sys

for p in ("/opt/trn_rl_repo",):
    if p not in sys.path:
        sys.path.insert(0, p)

from contextlib import ExitStack

import ml_dtypes
import numpy as np

import concourse.bass as bass
import concourse.mybir as mybir
import concourse.tile as tile
from concourse import bacc
from concourse.bass_utils import run_bass_kernel_spmd

# Problem shape (hardcoded per contest rules)
B, C, S, H = 2, 1024, 2048, 16
D = C // H  # 64
N_CORES = 8
HG = 4  # head groups
HPG = H // HG  # heads per group = 4
CPG = HPG * D  # channels per group = 256
P = 128
NK = C // P  # 8 contraction tiles over C
NST = S // P  # 16 key tiles
WIN = 1024  # q window
NWIN = S // WIN  # 2
NCH = WIN // 512  # 512-chunks per window = 2

F32 = mybir.dt.float32
BF16 = mybir.dt.bfloat16
EXP = mybir.ActivationFunctionType.Exp

_CACHED_NC = None


def build_nc():
    nc = bacc.Bacc("TRN2", target_bir_lowering=False, debug=False)

    x_d = nc.dram_tensor("x", (P, NK, S), BF16, kind="ExternalInput")
    wq_d = nc.dram_tensor("wqT", (P, NK, CPG), BF16, kind="ExternalInput")
    wk_d = nc.dram_tensor("wkT", (P, NK, CPG), BF16, kind="ExternalInput")
    wv_d = nc.dram_tensor("wvT", (P, NK, CPG), BF16, kind="ExternalInput")
    wo_d = nc.dram_tensor("woT", (P, 2, C), BF16, kind="ExternalInput")
    bq_d = nc.dram_tensor("bq", (P, 2), F32, kind="ExternalInput")
    out_d = nc.dram_tensor("out", (P, NK, S), F32, kind="ExternalOutput")

    with tile.TileContext(nc) as tc, ExitStack() as ctx:
        const = ctx.enter_context(tc.tile_pool(name="const", bufs=1))
        work = ctx.enter_context(tc.tile_pool(name="work", bufs=1))
        expp = ctx.enter_context(tc.tile_pool(name="expp", bufs=8))
        onp = ctx.enter_context(tc.tile_pool(name="onp", bufs=4))
        outp = ctx.enter_context(tc.tile_pool(name="outp", bufs=3))
        smal = ctx.enter_context(tc.tile_pool(name="smal", bufs=4))
        psc = ctx.enter_context(tc.tile_pool(name="psc", bufs=2, space="PSUM"))
        ppv = ctx.enter_context(tc.tile_pool(name="ppv", bufs=2, space="PSUM"))

        # ---- inputs in SBUF; x split per k-tile so DMA pipelines ----
        xt = []
        for kt in range(NK):
            t = const.tile([P, S], BF16, tag=f"x{kt}", name=f"x{kt}")
            nc.sync.dma_start(t[:], x_d[:, kt, :])
            xt.append(t)
        wq_sb = const.tile([P, NK, CPG], BF16, tag="wq")
        nc.sync.dma_start(wq_sb[:], wq_d[:])
        wk_sb = const.tile([P, NK, CPG], BF16, tag="wk")
        nc.sync.dma_start(wk_sb[:], wk_d[:])
        wv_sb = const.tile([P, NK, CPG], BF16, tag="wv")
        nc.sync.dma_start(wv_sb[:], wv_d[:])
        wo_sb = const.tile([P, 2, C], BF16, tag="wo")
        nc.sync.dma_start(wo_sb[:], wo_d[:])
        bq_sb = const.tile([P, 2], F32, tag="bq")
        nc.sync.dma_start(bq_sb[:], bq_d[:])

        # activations, one tile per (pair, window/chunk) for fine deps
        def wtile(nm):
            return work.tile([P, WIN], BF16, tag=nm, name=nm)

        k_t = [[wtile(f"k{p_}c{c}") for c in range(2)] for p_ in range(2)]
        q_t = [[wtile(f"q{p_}w{w}") for w in range(2)] for p_ in range(2)]
        o_t = [[wtile(f"o{p_}w{w}") for w in range(2)] for p_ in range(2)]
        vt = [
            work.tile([P, HPG, D + 1], BF16, tag=f"vt{st}", name=f"vt{st}")
            for st in range(NST)
        ]

        def qk_unit(w_sb, dst, pair, c, bias):
            ps = psc.tile([P, WIN], F32, tag="psc", name="ps_qk")
            for kt in range(NK):
                for ch in range(NCH):
                    nc.tensor.matmul(
                        ps[:, ch * 512 : (ch + 1) * 512],
                        w_sb[:, kt, pair * P : (pair + 1) * P],
                        xt[kt][:, c * WIN + ch * 512 : c * WIN + (ch + 1) * 512],
                        start=(kt == 0),
                        stop=(kt == NK - 1),
                    )
            if bias:
                nc.vector.tensor_scalar_add(
                    dst[:], ps[:], bq_sb[:, pair : pair + 1]
                )
            else:
                nc.vector.tensor_copy(dst[:], ps[:])

        def vt_unit(st):
            nc.vector.memset(vt[st][:], 1.0)
            ps = psc.tile([P, WIN], F32, tag="psc", name="ps_vt")
            for kt in range(NK):
                nc.tensor.matmul(
                    ps[:, :CPG],
                    xt[kt][:, st * P : (st + 1) * P],
                    wv_sb[:, kt, :],
                    start=(kt == 0),
                    stop=(kt == NK - 1),
                )
            nc.vector.tensor_copy(
                vt[st][:, :, 0:D],
                ps[:, :CPG].rearrange("p (h d) -> p h d", h=HPG),
            )

        def attention(w, pair, inject=None):
            inject = inject or {}
            oa = ppv.tile([P, WIN], F32, tag="ppv", name="oa")
            ob = ppv.tile([P, WIN], F32, tag="ppv", name="ob")

            def pv(prev):
                pkt, pea, peb = prev
                for ch in range(NCH):
                    cs = slice(ch * 512, (ch + 1) * 512)
                    nc.tensor.matmul(
                        oa[0 : D + 1, cs],
                        vt[pkt][:, 2 * pair, :],
                        pea[:, cs],
                        start=(pkt == 0),
                        stop=(pkt == NST - 1),
                    )
                    nc.tensor.matmul(
                        ob[0 : D + 1, cs],
                        vt[pkt][:, 2 * pair + 1, :],
                        peb[:, cs],
                        start=(pkt == 0),
                        stop=(pkt == NST - 1),
                    )

            prev = None
            for kt in range(NST):
                sa = psc.tile([P, WIN], F32, tag="psc", name="sa")
                sb = psc.tile([P, WIN], F32, tag="psc", name="sb")
                c, j = divmod(kt, NK)
                for ch in range(NCH):
                    cs = slice(ch * 512, (ch + 1) * 512)
                    nc.tensor.matmul(
                        sa[:, cs],
                        k_t[pair][c][0:D, j * P : (j + 1) * P],
                        q_t[pair][w][0:D, cs],
                        tile_position=(0, 0),
                    )
                    nc.tensor.matmul(
                        sb[:, cs],
                        k_t[pair][c][D:P, j * P : (j + 1) * P],
                        q_t[pair][w][D:P, cs],
                        tile_position=(64, 0),
                    )
                ea = expp.tile([P, WIN], BF16, tag="exp", name="ea")
                eb = expp.tile([P, WIN], BF16, tag="exp", name="eb")
                nc.scalar.activation(ea[:], sa[:], EXP, scale=float(D) ** -0.5)
                nc.scalar.activation(eb[:], sb[:], EXP, scale=float(D) ** -0.5)
                if prev is not None:
                    pv(prev)
                prev = (kt, ea, eb)
                for f in inject.get(kt, ()):
                    f()
            pv(prev)

            # fast evac to SBUF (frees PSUM), normalize off the hot path
            for head, acc in ((0, oa), (1, ob)):
                ou = onp.tile([D + 1, WIN], F32, tag="ou", name="ou")
                nc.vector.tensor_copy(ou[:], acc[0 : D + 1, :])
                rc = smal.tile([1, WIN], F32, tag="rc", name="rc")
                nc.vector.reciprocal(rc[:], ou[D : D + 1, :])
                rcb = smal.tile([D, WIN], F32, tag="rcb", name="rcb")
                nc.gpsimd.partition_broadcast(rcb[:], rc[:])
                nc.vector.tensor_mul(
                    o_t[pair][w][head * D : (head + 1) * D, :], ou[0:D, :], rcb[:]
                )

        def outproj(w):
            for m in range(NK):
                ps = ppv.tile([P, WIN], F32, tag="ppv", name="ps_out")
                for kt in range(2):
                    for ch in range(NCH):
                        cs = slice(ch * 512, (ch + 1) * 512)
                        nc.tensor.matmul(
                            ps[:, cs],
                            wo_sb[:, kt, m * P : (m + 1) * P],
                            o_t[kt][w][:, cs],
                            start=(kt == 0),
                            stop=(kt == 1),
                        )
                ot = outp.tile([P, WIN], F32, tag="ot", name="ot")
                nc.vector.tensor_copy(ot[:], ps[:])
                nc.sync.dma_start(out_d[:, m, w * WIN : (w + 1) * WIN], ot[:])

        # ---- emission schedule ----
        qk_unit(wk_sb, k_t[0][0], 0, 0, False)
        qk_unit(wq_sb, q_t[0][0], 0, 0, True)
        for st in range(4):
            vt_unit(st)

        def U(f, *a):
            return lambda: f(*a)

        inj00 = {
            1: (U(vt_unit, 4), U(vt_unit, 5)),
            2: (U(vt_unit, 6), U(vt_unit, 7)),
            3: (U(qk_unit, wk_sb, k_t[0][1], 0, 1, False),),
            4: (U(vt_unit, 8), U(vt_unit, 9)),
            5: (U(vt_unit, 10), U(vt_unit, 11)),
            6: (U(vt_unit, 12), U(vt_unit, 13)),
            7: (U(vt_unit, 14), U(vt_unit, 15)),
            8: (U(qk_unit, wk_sb, k_t[1][0], 1, 0, False),),
            10: (U(qk_unit, wq_sb, q_t[1][0], 1, 0, True),),
            12: (U(qk_unit, wk_sb, k_t[1][1], 1, 1, False),),
            14: (U(qk_unit, wq_sb, q_t[0][1], 0, 1, True),),
        }
        attention(0, 0, inj00)
        inj01 = {2: (U(qk_unit, wq_sb, q_t[1][1], 1, 1, True),)}
        attention(0, 1, inj01)
        attention(1, 0)
        outproj(0)
        attention(1, 1)
        outproj(1)

    nc.compile()
    return nc


def _shard_inputs(hidden_states, Wq, bq, Wk, Wv, bv, Wo, bo):
    bf = ml_dtypes.bfloat16
    in_maps = []
    for core in range(N_CORES):
        b, hg = divmod(core, HG)
        x = hidden_states[b, :, 0, :]  # (C, S) f32
        cs = slice(hg * CPG, (hg + 1) * CPG)
        wqT = Wq[cs, :].T.reshape(NK, P, CPG).transpose(1, 0, 2)
        wkT = Wk[cs, :].T.reshape(NK, P, CPG).transpose(1, 0, 2)
        wvT = Wv[cs, :].T.reshape(NK, P, CPG).transpose(1, 0, 2)
        woT = Wo[:, cs].T.reshape(2, P, C).transpose(1, 0, 2)
        in_maps.append(
            {
                "x": np.ascontiguousarray(
                    x.reshape(NK, P, S).transpose(1, 0, 2)
                ).astype(bf),
                "wqT": np.ascontiguousarray(wqT).astype(bf),
                "wkT": np.ascontiguousarray(wkT).astype(bf),
                "wvT": np.ascontiguousarray(wvT).astype(bf),
                "woT": np.ascontiguousarray(woT).astype(bf),
                "bq": np.ascontiguousarray(
                    bq[cs].reshape(2, P).T
                ).astype(np.float32),
            }
        )
    return in_maps


def get_nc():
    global _CACHED_NC
    if _CACHED_NC is None:
        _CACHED_NC = build_nc()
    return _CACHED_NC


def run(hidden_states, Wq, bq, Wk, Wv, bv, Wo, bo, trace=False, **kw):
    nc = get_nc()
    in_maps = _shard_inputs(hidden_states, Wq, bq, Wk, Wv, bv, Wo, bo)
    res = run_bass_kernel_spmd(
        nc, in_maps, core_ids=list(range(N_CORES)), trace=trace, **kw
    )
    # unshard: sum partials per batch, add host-side constant bias
    bias_vec = (Wo.astype(np.float64) @ bv.astype(np.float64)).astype(
        np.float32
    ) + bo
    out = np.zeros((B, C, 1, S), dtype=np.float32)
    for core in range(N_CORES):
        b = core // HG
        part = np.asarray(res.results[core]["out"], dtype=np.float32)
        out[b, :, 0, :] += part.transpose(1, 0, 2).reshape(C, S)
    out[:, :, 0, :] += bias_vec[None, :, None]
    return out, res


def kernel(**inputs):
    out, _ = run(**inputs)
    return out
